# revision 27
# baseline (speedup 1.0000x reference)
"""Trainium2 Bass kernel for CAttention:
    k      = einsum('bcit,i->bct', x, alpha)
    scores = einsum('bct,ts,bds->bcd', k, Wc, k)
    att    = softmax(scores, axis=-1)
    out    = einsum('bci,bint->bcnt', att, x)

Sharding: data-parallel over batch B=64 across 8 NeuronCores (8 batches/core).

Memory-bound: per-core traffic is x in (16.8MB fp16) + out (16.8MB fp16).
Measured DMA model: each HWDGE ring (SP-issued, ACT-issued) sustains
~190-240 GB/s alone; both together reach the ~358 GB/s per-core HBM cap.
=> floor ~94us when in (SP ring) and out (ACT ring) overlap; startup and
drain alternate rings so single-stream phases also run at ~358.

v3 vs the 158us v1 baseline:
  * bd(b) is computed ONE iteration before mix(b): the smalls chain
    (kT->V->scores->exp->recip->att->aT->er, ~12 cross-engine hops) hides
    under the previous batch's 16 mix matmuls instead of gating its own.
  * tree keeps fp16 through L3 (2x DVE mode; L4+ fp32) - DVE k-path drops
    from 12.2 to ~10.7us/batch.  rel-err stays ~1e-2 < 2e-2 gate.
  * PSUM evacuation in 1024-elem units (2 banks, 2 matmuls each), all on
    ACT in steady state; the last two batches split 4/4 with DVE (idle
    there) and the final batch's out-DMA quarters alternate rings.
  * X arrives as 2x 1MB halves per batch (16KB/partition contiguous rows;
    quarters across both rings for batch 0), prefetched 4 batches deep
    (bufs=6) so the in-stream start is never gated by slot recycling.
  * tc.tile_set_cur_wait pins each iteration to its own slot in the Tile
    scheduler's sim: without this the scheduler hoists 2-3 later k-paths
    ahead of the urgent chain ops and starves PE for ~20us at a time.

Per-core layout as v1: X fp16 [128, 8192], partition p = j*8+d, free
n2*64+t (n = j*128+n2); mix = block-diag(att^T) fp16 stationary.
"""

import sys

for _p in ("/opt/trn_rl_repo",):
    if _p not in sys.path:
        sys.path.insert(0, _p)

import numpy as np

B, C, N, T = 64, 8, 2048, 64
NCORES = 8
BS = B // NCORES          # batches per core
J = 16                    # n-chunks on partitions
N2 = N // J               # 128, n-extent in free dim
P = J * C                 # 128 partitions
F = N2 * T                # 8192 free elems
FH = F // 2
FQ = F // 4
UW = 1024                 # evac unit width (2 PSUM banks, 2 matmuls)

_PROGRAM_CACHE = {}


def _build_program():
    from contextlib import ExitStack

    import concourse.bacc as bacc
    from concourse import mybir, tile

    fp32 = mybir.dt.float32
    fp16 = mybir.dt.float16
    nc = bacc.Bacc("TRN2", target_bir_lowering=False, debug=False)

    xs = nc.dram_tensor("xs", [BS, C, N, T], fp16, kind="ExternalInput").ap()
    ac = nc.dram_tensor("ac", [P, N2], fp16, kind="ExternalInput").ap()
    # packed fp32: sel[0:8] | wcT[8:72] (rows 0-63) | id8[72:80] (rows 0-7) |
    #              rep[80:208] (rows 0-7) | mask[208:336]
    aux = nc.dram_tensor("aux", [P, 336], fp32, kind="ExternalInput").ap()
    out = nc.dram_tensor("out", [BS, C, N, T], fp16, kind="ExternalOutput").ap()

    Exp = mybir.ActivationFunctionType.Exp
    Copy = mybir.ActivationFunctionType.Copy
    ADD = mybir.AluOpType.add
    MULT = mybir.AluOpType.mult

    with tile.TileContext(nc) as tc, ExitStack() as ctx:
        cpool = ctx.enter_context(tc.tile_pool(name="const", bufs=1))
        xpool = ctx.enter_context(tc.tile_pool(name="x", bufs=6))
        apool = ctx.enter_context(tc.tile_pool(name="prod", bufs=2))
        scrpool = ctx.enter_context(tc.tile_pool(name="scr", bufs=2))
        opool = ctx.enter_context(tc.tile_pool(name="o", bufs=4))
        spool = ctx.enter_context(tc.tile_pool(name="small", bufs=2))
        bdpool = ctx.enter_context(tc.tile_pool(name="bd", bufs=3))
        mixp = ctx.enter_context(tc.tile_pool(name="mixp", bufs=3, space="PSUM"))
        psmall = ctx.enter_context(tc.tile_pool(name="psmall", bufs=2, space="PSUM"))

        ac_t = cpool.tile([P, N2], fp16)
        acf_t = cpool.tile([P, F], fp16)  # alpha replicated over t, on-device
        aux_t = cpool.tile([P, 336], fp32)
        sel_t = aux_t[:, 0:8]
        wcT_t = aux_t[:T, 8:72]
        id8_t = aux_t[:C, 72:80]
        rep_t = aux_t[:C, 80:208]
        mask_t = aux_t[:, 208:336]

        def xb_of(b):
            return xs[b].rearrange("d (j n2) t -> j d (n2 t)", j=J)

        def _acf_build_q(q):
            nc.vector.tensor_scalar_add(
                acf_t[:, q * FQ : (q + 1) * FQ].rearrange("p (n2 t) -> p n2 t", t=T),
                ac_t[:, q * (N2 // 4) : (q + 1) * (N2 // 4)]
                .rearrange("p (x n2) -> p n2 x", x=1)
                .to_broadcast([P, N2 // 4, T]),
                0.0,
            )

        def mult_q(prod, X, q):
            sl = slice(q * FQ, (q + 1) * FQ)
            nc.vector.tensor_tensor(
                out=prod[:, sl], in0=X[:, sl], in1=acf_t[:, sl], op=MULT
            )

        def tree_a(prod):
            """L1..L3: fp16 in-place folds (2x DVE mode)."""
            w = F // 2
            while w >= 1024:
                nc.vector.tensor_tensor(
                    out=prod[:, :w], in0=prod[:, :w], in1=prod[:, w : 2 * w], op=ADD
                )
                w //= 2

        def tree_b(prod):
            """L4..L7: fp32.  Returns scr with s = scr[:, :T]."""
            scr = scrpool.tile([P, 512], fp32, tag="scr")
            nc.vector.tensor_tensor(
                out=scr[:], in0=prod[:, :512], in1=prod[:, 512:1024], op=ADD
            )
            w = 256
            while w >= T:
                nc.vector.tensor_tensor(
                    out=scr[:, :w], in0=scr[:, :w], in1=scr[:, w : 2 * w], op=ADD
                )
                w //= 2
            return scr

        def dve_copy(dst, src):
            nc.vector.tensor_scalar_add(dst, src, 0.0)

        # ---- smalls chain pieces (PE matmul / DVE copy split) ----
        def kT_mm(scr):
            kT_ps = psmall.tile([T, C], fp32, tag="ps")
            nc.tensor.matmul(kT_ps[:], lhsT=scr[:, :T], rhs=sel_t, start=True, stop=True)
            return kT_ps

        def kT_cp(kT_ps):
            kT_sb = spool.tile([T, C], fp32, tag="kTsb")
            dve_copy(kT_sb[:], kT_ps[:])
            return kT_sb

        def v_mm(kT_sb):
            v_ps = psmall.tile([T, C], fp32, tag="ps")
            nc.tensor.matmul(v_ps[:], lhsT=wcT_t, rhs=kT_sb[:], start=True, stop=True)
            return v_ps

        def v_cp(v_ps):
            v_sb = spool.tile([T, C], fp32, tag="vsb")
            dve_copy(v_sb[:], v_ps[:])
            return v_sb

        def sc_exp(kT_sb, v_sb):
            sc_ps = psmall.tile([C, C], fp32, tag="ps")
            nc.tensor.matmul(sc_ps[:], lhsT=kT_sb[:], rhs=v_sb[:], start=True, stop=True)
            e_sb = spool.tile([C, C], fp32, tag="esb")
            ssum = spool.tile([C, 1], fp32, tag="ssum")
            nc.scalar.activation(e_sb[:], sc_ps[:], Exp, accum_out=ssum[:])
            return e_sb, ssum

        def recip_of(ssum):
            rcp = spool.tile([C, 1], fp32, tag="rcp")
            nc.vector.reciprocal(rcp[:], ssum[:])
            return rcp

        def att_aT_mm(e_sb, rcp):
            att_sb = spool.tile([C, C], fp32, tag="attsb")
            nc.scalar.activation(att_sb[:], e_sb[:], Copy, scale=rcp[:])
            aT_ps = psmall.tile([C, C], fp32, tag="ps")
            nc.tensor.transpose(aT_ps[:], att_sb[:], id8_t)
            return aT_ps

        def aT_cp(aT_ps):
            aT_sb = spool.tile([C, C], fp32, tag="aTsb")
            dve_copy(aT_sb[:], aT_ps[:])
            return aT_sb

        def er_mm(aT_sb):
            er_ps = psmall.tile([P, C], fp32, tag="ps")
            nc.tensor.matmul(er_ps[:], lhsT=rep_t, rhs=aT_sb[:], start=True, stop=True)
            return er_ps

        def emit_bd(er_ps):
            bd = bdpool.tile([P, P], fp16, tag="bd")
            nc.vector.tensor_tensor(
                out=bd[:].rearrange("p (j c) -> p j c", j=J),
                in0=mask_t.rearrange("p (j c) -> p j c", j=J),
                in1=er_ps[:].rearrange("p (x c) -> p x c", x=1).to_broadcast([P, J, C]),
                op=MULT,
            )
            return bd

        def mix_unit(X, bd, ost, u, on_dve):
            mp = mixp.tile([P, UW], fp32, tag="mix")
            for h in range(2):
                lo = u * UW + h * 512
                nc.tensor.matmul(
                    mp[:, h * 512 : (h + 1) * 512],
                    lhsT=bd[:], rhs=X[:, lo : lo + 512],
                    start=True, stop=True,
                )
            osl = ost[:, (u % 4) * UW : (u % 4 + 1) * UW]
            if on_dve:
                nc.vector.tensor_scalar_add(osl, mp[:], 0.0)
            else:
                nc.scalar.activation(osl, mp[:], Copy)

        # ================= startup =================
        # batch 0: X quarters alternate SP/ACT rings; acf built between
        nc.scalar.dma_start(ac_t[:], ac)
        Xt = {}
        Xt[0] = xpool.tile([P, F], fp16, tag="X", name="Xv")
        xb0 = xb_of(0)
        for q in range(4):
            sl = slice(q * FQ, (q + 1) * FQ)
            if q % 2 == 0:
                nc.sync.dma_start(Xt[0][:, sl], xb0[:, :, sl])
            else:
                nc.scalar.dma_start(Xt[0][:, sl], xb0[:, :, sl])
            _acf_build_q(q)
        nc.scalar.dma_start(aux_t[:], aux)

        Xt[1] = xpool.tile([P, F], fp16, tag="X", name="Xv")
        xb1 = xb_of(1)
        nc.sync.dma_start(Xt[1][:, :FH], xb1[:, :, :FH])
        nc.scalar.dma_start(Xt[1][:, FH:], xb1[:, :, FH:])
        Xt[2] = xpool.tile([P, F], fp16, tag="X", name="Xv")
        xb2 = xb_of(2)
        nc.sync.dma_start(Xt[2][:, :FH], xb2[:, :, :FH])
        nc.sync.dma_start(Xt[2][:, FH:], xb2[:, :, FH:])
        Xt[3] = xpool.tile([P, F], fp16, tag="X", name="Xv")
        xb3s = xb_of(3)
        nc.sync.dma_start(Xt[3][:, :FH], xb3s[:, :, :FH])
        nc.sync.dma_start(Xt[3][:, FH:], xb3s[:, :, FH:])

        # k-path(0), then chain(0) interleaved with k-path(1)
        prod0 = apool.tile([P, F], fp16, tag="prod")
        for q in range(4):
            mult_q(prod0, Xt[0], q)
        tree_a(prod0)
        scr0 = tree_b(prod0)

        kT_ps_ = kT_mm(scr0)
        prod1 = apool.tile([P, F], fp16, tag="prod")
        mult_q(prod1, Xt[1], 0)
        kT_sb = kT_cp(kT_ps_)
        v_ps_ = v_mm(kT_sb)
        mult_q(prod1, Xt[1], 1)
        v_sb = v_cp(v_ps_)
        e_sb, ssum = sc_exp(kT_sb, v_sb)
        mult_q(prod1, Xt[1], 2)
        rcp = recip_of(ssum)
        aT_ps_ = att_aT_mm(e_sb, rcp)
        mult_q(prod1, Xt[1], 3)
        aT_sb = aT_cp(aT_ps_)
        er_ps_ = er_mm(aT_sb)
        tree_a(prod1)
        bd = emit_bd(er_ps_)
        scr = tree_b(prod1)

        # ================= steady loop =================
        # iteration b: mix(b) [bd(b) ready], smalls(b+1) interleaved between
        # mix units, k-path(b+2) on DVE [X(b+2) already resident], X(b+4) in.
        # tile_set_cur_wait pins each iteration to its own scheduler-sim slot
        # so the scheduler cannot hoist later k-paths ahead of the urgent
        # chain ops (observed: 3 batches of DVE run-ahead starving PE).
        for b in range(BS):
            tc.tile_set_cur_wait(0.050 + b * 0.030)
            last = b == BS - 1
            do_next = not last          # smalls/bd for b+1
            do_next2 = b + 2 < BS       # k-path for b+2

            if b + 4 < BS:
                Xt[b + 4] = xpool.tile([P, F], fp16, tag="X", name="Xv")
                xb4 = xb_of(b + 4)
                nc.sync.dma_start(Xt[b + 4][:, :FH], xb4[:, :, :FH])
                nc.sync.dma_start(Xt[b + 4][:, FH:], xb4[:, :, FH:])
            X = Xt[b]
            Xn2 = Xt[b + 2] if do_next2 else None
            if do_next2:
                prodn = apool.tile([P, F], fp16, tag="prod")

            out_b = out[b].rearrange("c (j n2) t -> j c (n2 t)", j=J)
            dve_units = (1, 3, 5, 7) if b >= BS - 2 else ()
            # iteration 0: mix(0) leads PE's stream (chain(1) would wait on
            # tree(1), which only lands mid-mix(0) during startup)
            chain_late = b == 0

            ost = opool.tile([P, FH], fp16, tag="ost")
            if do_next and not chain_late:
                kT_ps_ = kT_mm(scr)
            mix_unit(X, bd, ost, 0, on_dve=0 in dve_units)
            if do_next2:
                mult_q(prodn, Xn2, 0)
            if do_next and not chain_late:
                kT_sb = kT_cp(kT_ps_)
                v_ps_ = v_mm(kT_sb)
            mix_unit(X, bd, ost, 1, on_dve=1 in dve_units)
            if do_next2:
                mult_q(prodn, Xn2, 1)
            if do_next and not chain_late:
                v_sb = v_cp(v_ps_)
                e_sb, ssum = sc_exp(kT_sb, v_sb)
            mix_unit(X, bd, ost, 2, on_dve=2 in dve_units)
            if do_next2:
                mult_q(prodn, Xn2, 2)
            if do_next and not chain_late:
                rcp = recip_of(ssum)
            mix_unit(X, bd, ost, 3, on_dve=3 in dve_units)
            if do_next and chain_late:
                kT_ps_ = kT_mm(scr)
                kT_sb = kT_cp(kT_ps_)
                v_ps_ = v_mm(kT_sb)
                v_sb = v_cp(v_ps_)
                e_sb, ssum = sc_exp(kT_sb, v_sb)
                rcp = recip_of(ssum)
            if last:
                nc.scalar.dma_start(out_b[:, :, 0:FQ], ost[:, :FQ])
                nc.sync.dma_start(out_b[:, :, FQ:FH], ost[:, FQ:])
            else:
                nc.scalar.dma_start(out_b[:, :, 0:FH], ost[:])

            ost = opool.tile([P, FH], fp16, tag="ost")
            if do_next and not chain_late:
                aT_ps_ = att_aT_mm(e_sb, rcp)
            mix_unit(X, bd, ost, 4, on_dve=4 in dve_units)
            if do_next2:
                mult_q(prodn, Xn2, 3)
            if do_next and not chain_late:
                aT_sb = aT_cp(aT_ps_)
                er_ps_ = er_mm(aT_sb)
            mix_unit(X, bd, ost, 5, on_dve=5 in dve_units)
            if do_next2:
                tree_a(prodn)
            if do_next and chain_late:
                aT_ps_ = att_aT_mm(e_sb, rcp)
                aT_sb = aT_cp(aT_ps_)
                er_ps_ = er_mm(aT_sb)
            if do_next and not chain_late:
                bd_next = emit_bd(er_ps_)
            mix_unit(X, bd, ost, 6, on_dve=6 in dve_units)
            if do_next and chain_late:
                bd_next = emit_bd(er_ps_)
            mix_unit(X, bd, ost, 7, on_dve=7 in dve_units)
            if do_next2:
                scr = tree_b(prodn)
            if last:
                nc.sync.dma_start(out_b[:, :, FH : FH + FQ], ost[:, :FQ])
                nc.scalar.dma_start(out_b[:, :, FH + FQ : F], ost[:, FQ:])
            else:
                nc.scalar.dma_start(out_b[:, :, FH:F], ost[:])

            if do_next:
                bd = bd_next

    nc.compile()
    return nc


def _host_constants(Wc: np.ndarray, alpha: np.ndarray):
    a = np.asarray(alpha, dtype=np.float16).reshape(J, N2)
    ac = np.repeat(a, C, axis=0)                         # [128, N2]
    sel = np.tile(np.eye(C, dtype=np.float32), (J, 1))
    id8 = np.eye(C, dtype=np.float32)
    rep = np.tile(np.eye(C, dtype=np.float32), (1, J))
    mask = np.kron(np.eye(J, dtype=np.float32), np.ones((C, C), dtype=np.float32))
    aux = np.zeros((P, 336), dtype=np.float32)
    aux[:, 0:8] = sel
    aux[:T, 8:72] = np.asarray(Wc.T, dtype=np.float32)
    aux[:C, 72:80] = id8
    aux[:C, 80:208] = rep
    aux[:, 208:336] = mask
    return {
        "ac": np.ascontiguousarray(ac),
        "aux": aux,
    }


def get_program():
    if "nc" not in _PROGRAM_CACHE:
        _PROGRAM_CACHE["nc"] = _build_program()
    return _PROGRAM_CACHE["nc"]


def run(x, Wc, alpha, trace=False, trace_kwargs=None):
    """Run on 8 cores; returns (full_output fp32, BassKernelResults)."""
    from concourse.bass_utils import run_bass_kernel_spmd

    nc = get_program()
    consts = _host_constants(np.asarray(Wc), np.asarray(alpha))
    x16 = np.asarray(x).astype(np.float16)
    in_maps = []
    for r in range(NCORES):
        m = {"xs": np.ascontiguousarray(x16[r * BS : (r + 1) * BS])}
        m.update(consts)
        in_maps.append(m)
    kw = {}
    if trace:
        kw["trace"] = True
        if trace_kwargs:
            kw.update(trace_kwargs)
    res = run_bass_kernel_spmd(nc, in_maps, list(range(NCORES)), **kw)
    out = np.concatenate([res.results[r]["out"] for r in range(NCORES)], axis=0)
    return out.astype(np.float32), res


def kernel(x, Wc, alpha):
    out, _ = run(x, Wc, alpha)
    return out


# revision 28
# speedup vs baseline: 1.0646x; 1.0646x over previous
"""Trainium2 Bass kernel for CAttention:
    k      = einsum('bcit,i->bct', x, alpha)
    scores = einsum('bct,ts,bds->bcd', k, Wc, k)
    att    = softmax(scores, axis=-1)
    out    = einsum('bci,bint->bcnt', att, x)

Sharding: data-parallel over batch B=64 across 8 NeuronCores (8 batches/core).

Memory-bound: per-core traffic is x in (16.8MB fp16) + out (16.8MB fp16).
Measured DMA model: each HWDGE ring (SP-issued, ACT-issued) sustains
~190-240 GB/s alone; both together reach the ~358 GB/s per-core HBM cap.
=> floor ~94us when in (SP ring) and out (ACT ring) overlap; startup and
drain alternate rings so single-stream phases also run at ~358.

v3 vs the 158us v1 baseline:
  * bd(b) is computed ONE iteration before mix(b): the smalls chain
    (kT->V->scores->exp->recip->att->aT->er, ~12 cross-engine hops) hides
    under the previous batch's 16 mix matmuls instead of gating its own.
  * tree keeps fp16 through L3 (2x DVE mode; L4+ fp32) - DVE k-path drops
    from 12.2 to ~10.7us/batch.  rel-err stays ~1e-2 < 2e-2 gate.
  * PSUM evacuation in 1024-elem units (2 banks, 2 matmuls each), all on
    ACT in steady state; the last two batches split 4/4 with DVE (idle
    there) and the final batch's out-DMA quarters alternate rings.
  * X arrives as 2x 1MB halves per batch (16KB/partition contiguous rows;
    quarters across both rings for batch 0), prefetched 4 batches deep
    (bufs=6) so the in-stream start is never gated by slot recycling.
  * tc.tile_set_cur_wait pins each iteration to its own slot in the Tile
    scheduler's sim: without this the scheduler hoists 2-3 later k-paths
    ahead of the urgent chain ops and starves PE for ~20us at a time.

Per-core layout as v1: X fp16 [128, 8192], partition p = j*8+d, free
n2*64+t (n = j*128+n2); mix = block-diag(att^T) fp16 stationary.
"""

import sys

for _p in ("/opt/trn_rl_repo",):
    if _p not in sys.path:
        sys.path.insert(0, _p)

import numpy as np

B, C, N, T = 64, 8, 2048, 64
NCORES = 8
BS = B // NCORES          # batches per core
J = 16                    # n-chunks on partitions
N2 = N // J               # 128, n-extent in free dim
P = J * C                 # 128 partitions
F = N2 * T                # 8192 free elems
FH = F // 2
FQ = F // 4
UW = 1024                 # evac unit width (2 PSUM banks, 2 matmuls)

_PROGRAM_CACHE = {}


def _build_program():
    from contextlib import ExitStack

    import concourse.bacc as bacc
    from concourse import mybir, tile

    fp32 = mybir.dt.float32
    fp16 = mybir.dt.float16
    nc = bacc.Bacc("TRN2", target_bir_lowering=False, debug=False)

    xs = nc.dram_tensor("xs", [BS, C, N, T], fp16, kind="ExternalInput").ap()
    ac = nc.dram_tensor("ac", [P, N2], fp16, kind="ExternalInput").ap()
    # packed fp32: sel[0:8] | wcT[8:72] (rows 0-63) | id8[72:80] (rows 0-7) |
    #              rep[80:208] (rows 0-7) | mask[208:336]
    aux = nc.dram_tensor("aux", [P, 336], fp32, kind="ExternalInput").ap()
    out = nc.dram_tensor("out", [BS, C, N, T], fp16, kind="ExternalOutput").ap()

    Exp = mybir.ActivationFunctionType.Exp
    Copy = mybir.ActivationFunctionType.Copy
    ADD = mybir.AluOpType.add
    MULT = mybir.AluOpType.mult

    with tile.TileContext(nc) as tc, ExitStack() as ctx:
        cpool = ctx.enter_context(tc.tile_pool(name="const", bufs=1))
        xpool = ctx.enter_context(tc.tile_pool(name="x", bufs=6))
        apool = ctx.enter_context(tc.tile_pool(name="prod", bufs=2))
        scrpool = ctx.enter_context(tc.tile_pool(name="scr", bufs=2))
        opool = ctx.enter_context(tc.tile_pool(name="o", bufs=4))
        spool = ctx.enter_context(tc.tile_pool(name="small", bufs=2))
        bdpool = ctx.enter_context(tc.tile_pool(name="bd", bufs=3))
        mixp = ctx.enter_context(tc.tile_pool(name="mixp", bufs=3, space="PSUM"))
        psmall = ctx.enter_context(tc.tile_pool(name="psmall", bufs=2, space="PSUM"))

        ac_t = cpool.tile([P, N2], fp16)
        acf_t = cpool.tile([P, F], fp16)  # alpha replicated over t, on-device
        aux_t = cpool.tile([P, 336], fp32)
        sel_t = aux_t[:, 0:8]
        wcT_t = aux_t[:T, 8:72]
        id8_t = aux_t[:C, 72:80]
        rep_t = aux_t[:C, 80:208]
        mask_t = aux_t[:, 208:336]

        def xb_of(b):
            return xs[b].rearrange("d (j n2) t -> j d (n2 t)", j=J)

        def _acf_build_q(q):
            nc.vector.tensor_scalar_add(
                acf_t[:, q * FQ : (q + 1) * FQ].rearrange("p (n2 t) -> p n2 t", t=T),
                ac_t[:, q * (N2 // 4) : (q + 1) * (N2 // 4)]
                .rearrange("p (x n2) -> p n2 x", x=1)
                .to_broadcast([P, N2 // 4, T]),
                0.0,
            )

        def mult_q(prod, X, q):
            sl = slice(q * FQ, (q + 1) * FQ)
            nc.vector.tensor_tensor(
                out=prod[:, sl], in0=X[:, sl], in1=acf_t[:, sl], op=MULT
            )

        def tree_a(prod):
            """L1..L3: fp16 in-place folds (2x DVE mode)."""
            w = F // 2
            while w >= 1024:
                nc.vector.tensor_tensor(
                    out=prod[:, :w], in0=prod[:, :w], in1=prod[:, w : 2 * w], op=ADD
                )
                w //= 2

        def tree_b(prod):
            """L4..L7: fp32.  Returns scr with s = scr[:, :T]."""
            scr = scrpool.tile([P, 512], fp32, tag="scr")
            nc.vector.tensor_tensor(
                out=scr[:], in0=prod[:, :512], in1=prod[:, 512:1024], op=ADD
            )
            w = 256
            while w >= T:
                nc.vector.tensor_tensor(
                    out=scr[:, :w], in0=scr[:, :w], in1=scr[:, w : 2 * w], op=ADD
                )
                w //= 2
            return scr

        def dve_copy(dst, src):
            nc.vector.tensor_scalar_add(dst, src, 0.0)

        # ---- smalls chain pieces (PE matmul / DVE copy split) ----
        def kT_mm(scr):
            kT_ps = psmall.tile([T, C], fp32, tag="ps")
            nc.tensor.matmul(kT_ps[:], lhsT=scr[:, :T], rhs=sel_t, start=True, stop=True)
            return kT_ps

        def kT_cp(kT_ps):
            kT_sb = spool.tile([T, C], fp32, tag="kTsb")
            dve_copy(kT_sb[:], kT_ps[:])
            return kT_sb

        def v_mm(kT_sb):
            v_ps = psmall.tile([T, C], fp32, tag="ps")
            nc.tensor.matmul(v_ps[:], lhsT=wcT_t, rhs=kT_sb[:], start=True, stop=True)
            return v_ps

        def v_cp(v_ps):
            v_sb = spool.tile([T, C], fp32, tag="vsb")
            dve_copy(v_sb[:], v_ps[:])
            return v_sb

        def sc_exp(kT_sb, v_sb):
            sc_ps = psmall.tile([C, C], fp32, tag="ps")
            nc.tensor.matmul(sc_ps[:], lhsT=kT_sb[:], rhs=v_sb[:], start=True, stop=True)
            e_sb = spool.tile([C, C], fp32, tag="esb")
            ssum = spool.tile([C, 1], fp32, tag="ssum")
            nc.scalar.activation(e_sb[:], sc_ps[:], Exp, accum_out=ssum[:])
            return e_sb, ssum

        def recip_of(ssum):
            rcp = spool.tile([C, 1], fp32, tag="rcp")
            nc.vector.reciprocal(rcp[:], ssum[:])
            return rcp

        def att_aT_mm(e_sb, rcp):
            att_sb = spool.tile([C, C], fp32, tag="attsb")
            nc.scalar.activation(att_sb[:], e_sb[:], Copy, scale=rcp[:])
            aT_ps = psmall.tile([C, C], fp32, tag="ps")
            nc.tensor.transpose(aT_ps[:], att_sb[:], id8_t)
            return aT_ps

        def aT_cp(aT_ps):
            aT_sb = spool.tile([C, C], fp32, tag="aTsb")
            dve_copy(aT_sb[:], aT_ps[:])
            return aT_sb

        def er_mm(aT_sb):
            er_ps = psmall.tile([P, C], fp32, tag="ps")
            nc.tensor.matmul(er_ps[:], lhsT=rep_t, rhs=aT_sb[:], start=True, stop=True)
            return er_ps

        def emit_bd(er_ps):
            bd = bdpool.tile([P, P], fp16, tag="bd")
            nc.vector.tensor_tensor(
                out=bd[:].rearrange("p (j c) -> p j c", j=J),
                in0=mask_t.rearrange("p (j c) -> p j c", j=J),
                in1=er_ps[:].rearrange("p (x c) -> p x c", x=1).to_broadcast([P, J, C]),
                op=MULT,
            )
            return bd

        def mix_unit(X, bd, ost, u, on_dve):
            mp = mixp.tile([P, UW], fp32, tag="mix")
            for h in range(2):
                lo = u * UW + h * 512
                nc.tensor.matmul(
                    mp[:, h * 512 : (h + 1) * 512],
                    lhsT=bd[:], rhs=X[:, lo : lo + 512],
                    start=True, stop=True,
                )
            osl = ost[:, (u % 4) * UW : (u % 4 + 1) * UW]
            if on_dve:
                nc.vector.tensor_scalar_add(osl, mp[:], 0.0)
            else:
                nc.scalar.activation(osl, mp[:], Copy)

        # ================= startup =================
        # batch 0: X quarters alternate SP/ACT rings; acf built between
        nc.scalar.dma_start(ac_t[:], ac)
        Xt = {}
        Xt[0] = xpool.tile([P, F], fp16, tag="X", name="Xv")
        xb0 = xb_of(0)
        for q in range(4):
            sl = slice(q * FQ, (q + 1) * FQ)
            if q % 2 == 0:
                nc.sync.dma_start(Xt[0][:, sl], xb0[:, :, sl])
            else:
                nc.scalar.dma_start(Xt[0][:, sl], xb0[:, :, sl])
            _acf_build_q(q)
        nc.scalar.dma_start(aux_t[:], aux)

        Xt[1] = xpool.tile([P, F], fp16, tag="X", name="Xv")
        xb1 = xb_of(1)
        nc.sync.dma_start(Xt[1][:, :FH], xb1[:, :, :FH])
        nc.scalar.dma_start(Xt[1][:, FH:], xb1[:, :, FH:])
        Xt[2] = xpool.tile([P, F], fp16, tag="X", name="Xv")
        xb2 = xb_of(2)
        nc.sync.dma_start(Xt[2][:, :FH], xb2[:, :, :FH])
        nc.sync.dma_start(Xt[2][:, FH:], xb2[:, :, FH:])
        Xt[3] = xpool.tile([P, F], fp16, tag="X", name="Xv")
        xb3s = xb_of(3)
        nc.sync.dma_start(Xt[3][:, :FH], xb3s[:, :, :FH])
        nc.sync.dma_start(Xt[3][:, FH:], xb3s[:, :, FH:])

        # k-path(0), then chain(0) interleaved with k-path(1)
        prod0 = apool.tile([P, F], fp16, tag="prod")
        for q in range(4):
            mult_q(prod0, Xt[0], q)
        tree_a(prod0)
        scr0 = tree_b(prod0)

        kT_ps_ = kT_mm(scr0)
        prod1 = apool.tile([P, F], fp16, tag="prod")
        mult_q(prod1, Xt[1], 0)
        kT_sb = kT_cp(kT_ps_)
        v_ps_ = v_mm(kT_sb)
        mult_q(prod1, Xt[1], 1)
        v_sb = v_cp(v_ps_)
        e_sb, ssum = sc_exp(kT_sb, v_sb)
        mult_q(prod1, Xt[1], 2)
        rcp = recip_of(ssum)
        aT_ps_ = att_aT_mm(e_sb, rcp)
        mult_q(prod1, Xt[1], 3)
        aT_sb = aT_cp(aT_ps_)
        er_ps_ = er_mm(aT_sb)
        tree_a(prod1)
        bd = emit_bd(er_ps_)
        scr = tree_b(prod1)

        # ================= steady loop =================
        # iteration b: mix(b) [bd(b) ready], smalls(b+1) interleaved between
        # mix units, k-path(b+2) on DVE [X(b+2) already resident], X(b+4) in.
        # tile_set_cur_wait pins each iteration to its own scheduler-sim slot
        # so the scheduler cannot hoist later k-paths ahead of the urgent
        # chain ops (observed: 3 batches of DVE run-ahead starving PE).
        for b in range(BS):
            tc.tile_set_cur_wait(0.050 + b * 0.030)
            last = b == BS - 1
            do_next = not last          # smalls/bd for b+1
            do_next2 = b + 2 < BS       # k-path for b+2

            if b + 4 < BS:
                Xt[b + 4] = xpool.tile([P, F], fp16, tag="X", name="Xv")
                xb4 = xb_of(b + 4)
                nc.sync.dma_start(Xt[b + 4][:, :FH], xb4[:, :, :FH])
                nc.sync.dma_start(Xt[b + 4][:, FH:], xb4[:, :, FH:])
            X = Xt[b]
            Xn2 = Xt[b + 2] if do_next2 else None
            if do_next2:
                prodn = apool.tile([P, F], fp16, tag="prod")

            out_b = out[b].rearrange("c (j n2) t -> j c (n2 t)", j=J)
            dve_units = (1, 3, 5, 7) if b >= BS - 2 else ()
            chain_late = False

            ost = opool.tile([P, FH], fp16, tag="ost")
            if do_next and not chain_late:
                kT_ps_ = kT_mm(scr)
            mix_unit(X, bd, ost, 0, on_dve=0 in dve_units)
            if do_next2:
                mult_q(prodn, Xn2, 0)
            if do_next and not chain_late:
                kT_sb = kT_cp(kT_ps_)
                v_ps_ = v_mm(kT_sb)
            mix_unit(X, bd, ost, 1, on_dve=1 in dve_units)
            if do_next2:
                mult_q(prodn, Xn2, 1)
            if do_next and not chain_late:
                v_sb = v_cp(v_ps_)
                e_sb, ssum = sc_exp(kT_sb, v_sb)
            mix_unit(X, bd, ost, 2, on_dve=2 in dve_units)
            if do_next2:
                mult_q(prodn, Xn2, 2)
            if do_next and not chain_late:
                rcp = recip_of(ssum)
            mix_unit(X, bd, ost, 3, on_dve=3 in dve_units)
            if do_next and chain_late:
                kT_ps_ = kT_mm(scr)
                kT_sb = kT_cp(kT_ps_)
                v_ps_ = v_mm(kT_sb)
                v_sb = v_cp(v_ps_)
                e_sb, ssum = sc_exp(kT_sb, v_sb)
                rcp = recip_of(ssum)
            if last:
                nc.scalar.dma_start(out_b[:, :, 0:FQ], ost[:, :FQ])
                nc.sync.dma_start(out_b[:, :, FQ:FH], ost[:, FQ:])
            else:
                nc.scalar.dma_start(out_b[:, :, 0:FH], ost[:])

            ost = opool.tile([P, FH], fp16, tag="ost")
            if do_next and not chain_late:
                aT_ps_ = att_aT_mm(e_sb, rcp)
            mix_unit(X, bd, ost, 4, on_dve=4 in dve_units)
            if do_next2:
                mult_q(prodn, Xn2, 3)
            if do_next and not chain_late:
                aT_sb = aT_cp(aT_ps_)
                er_ps_ = er_mm(aT_sb)
            mix_unit(X, bd, ost, 5, on_dve=5 in dve_units)
            if do_next2:
                tree_a(prodn)
            if do_next and chain_late:
                aT_ps_ = att_aT_mm(e_sb, rcp)
                aT_sb = aT_cp(aT_ps_)
                er_ps_ = er_mm(aT_sb)
            if do_next and not chain_late:
                bd_next = emit_bd(er_ps_)
            mix_unit(X, bd, ost, 6, on_dve=6 in dve_units)
            if do_next and chain_late:
                bd_next = emit_bd(er_ps_)
            mix_unit(X, bd, ost, 7, on_dve=7 in dve_units)
            if do_next2:
                scr = tree_b(prodn)
            if last:
                nc.sync.dma_start(out_b[:, :, FH : FH + FQ], ost[:, :FQ])
                nc.scalar.dma_start(out_b[:, :, FH + FQ : F], ost[:, FQ:])
            else:
                nc.scalar.dma_start(out_b[:, :, FH:F], ost[:])

            if do_next:
                bd = bd_next

    nc.compile()
    return nc


def _host_constants(Wc: np.ndarray, alpha: np.ndarray):
    a = np.asarray(alpha, dtype=np.float16).reshape(J, N2)
    ac = np.repeat(a, C, axis=0)                         # [128, N2]
    sel = np.tile(np.eye(C, dtype=np.float32), (J, 1))
    id8 = np.eye(C, dtype=np.float32)
    rep = np.tile(np.eye(C, dtype=np.float32), (1, J))
    mask = np.kron(np.eye(J, dtype=np.float32), np.ones((C, C), dtype=np.float32))
    aux = np.zeros((P, 336), dtype=np.float32)
    aux[:, 0:8] = sel
    aux[:T, 8:72] = np.asarray(Wc.T, dtype=np.float32)
    aux[:C, 72:80] = id8
    aux[:C, 80:208] = rep
    aux[:, 208:336] = mask
    return {
        "ac": np.ascontiguousarray(ac),
        "aux": aux,
    }


def get_program():
    if "nc" not in _PROGRAM_CACHE:
        _PROGRAM_CACHE["nc"] = _build_program()
    return _PROGRAM_CACHE["nc"]


def run(x, Wc, alpha, trace=False, trace_kwargs=None):
    """Run on 8 cores; returns (full_output fp32, BassKernelResults)."""
    from concourse.bass_utils import run_bass_kernel_spmd

    nc = get_program()
    consts = _host_constants(np.asarray(Wc), np.asarray(alpha))
    x16 = np.asarray(x).astype(np.float16)
    in_maps = []
    for r in range(NCORES):
        m = {"xs": np.ascontiguousarray(x16[r * BS : (r + 1) * BS])}
        m.update(consts)
        in_maps.append(m)
    kw = {}
    if trace:
        kw["trace"] = True
        if trace_kwargs:
            kw.update(trace_kwargs)
    res = run_bass_kernel_spmd(nc, in_maps, list(range(NCORES)), **kw)
    out = np.concatenate([res.results[r]["out"] for r in range(NCORES)], axis=0)
    return out.astype(np.float32), res


def kernel(x, Wc, alpha):
    out, _ = run(x, Wc, alpha)
    return out
